# revision 4
# baseline (speedup 1.0000x reference)
"""
MultiHeadAttention (B=4, S=2048, D=512, H=8) on 8 trn2 NeuronCores.

Sharding: core c handles batch b=c//2 and 4 heads hs=(c%2)*4..+4
(data parallel on B, tensor parallel on H).

Device kernel (per core), all matmuls bf16 with f32 PSUM accumulation:
  A) LayerNorm q,k,v (bn_stats) -> transpose to [D, S] via PE
  B) Project: qhT,khT = W.T @ xnT  -> [dh=256, S] (head-transposed),
     vh = vn @ Wv -> [S, dv] natural, with a ones column appended per head
  C) Flash-style attention in transposed score layout:
     scoresT[k,q] = khT.T@qhT (PSUM) -> exp on ACT -> * expbiasT (host
     precomputed exp(pos_k).T * mask.T, bf16) -> attnU^T (bf16, stored to
     HBM unnormalized; host divides by sums) ; xT_plus = [vh|1].T @ attnU^T
     gives x^T rows + softmax sums row; normalize x^T with 1/sums.
  D) out_pT = wo_c.T @ xT_all -> [512, S] f32 partial (host sums the two
     head-halves, adds biases, applies layer_scale).
"""

import os
import sys

sys.path.insert(0, "/opt/trn_rl_repo")

import numpy as np
import ml_dtypes

import concourse.bass as bass
import concourse.bacc as bacc
import concourse.mybir as mybir
import concourse.tile as tile
from concourse.bass_utils import run_bass_kernel_spmd
from concourse.masks import make_identity

BF = mybir.dt.bfloat16
F32 = mybir.dt.float32
NPBF = ml_dtypes.bfloat16

B, S, D, H = 4, 2048, 512, 8
DK = D // H          # 64
HL = H // 2          # 4 heads per core
DL = HL * DK         # 256 local head dims
NQC = 4              # q chunks of 512
QW = S // NQC        # 512
NKT = S // 128       # 16 k tiles
NST = S // 128       # 16 s tiles
NDC = D // 128       # 4 d chunks

_built = None


def _build():
    """Build + compile the per-core Bass program (identical on all cores)."""
    nc = bacc.Bacc("TRN2", target_bir_lowering=False, debug=False, num_devices=8)

    xq = nc.dram_tensor("xq", [S, D], BF, kind="ExternalInput").ap()
    xk = nc.dram_tensor("xk", [S, D], BF, kind="ExternalInput").ap()
    xv = nc.dram_tensor("xv", [S, D], BF, kind="ExternalInput").ap()
    wq = nc.dram_tensor("wq", [128, NDC, DL], BF, kind="ExternalInput").ap()
    wk = nc.dram_tensor("wk", [128, NDC, DL], BF, kind="ExternalInput").ap()
    wv = nc.dram_tensor("wv", [128, NDC, DL], BF, kind="ExternalInput").ap()
    wo = nc.dram_tensor("wo", [128, 2, D], BF, kind="ExternalInput").ap()
    bq = nc.dram_tensor("bq", [128, 2], F32, kind="ExternalInput").ap()
    bk = nc.dram_tensor("bk", [128, 2], F32, kind="ExternalInput").ap()
    ebt = nc.dram_tensor("ebt", [S, S], BF, kind="ExternalInput").ap()  # [k, q]

    attnu = nc.dram_tensor("attnu", [HL, S, S], BF, kind="ExternalOutput").ap()  # [h,k,q]
    recs = nc.dram_tensor("recs", [HL, S], F32, kind="ExternalOutput").ap()      # 1/sums
    outp = nc.dram_tensor("outp", [D, S], F32, kind="ExternalOutput").ap()       # [dout,q]

    Exp = mybir.ActivationFunctionType.Exp
    Sqrt = mybir.ActivationFunctionType.Sqrt
    MUL = mybir.AluOpType.mult
    SUB = mybir.AluOpType.subtract
    ADD = mybir.AluOpType.add

    with tile.TileContext(nc) as tc:
        with tc.tile_pool(name="singles", bufs=1) as singles, \
             tc.tile_pool(name="pb", bufs=1) as pb:

            ident = singles.tile([128, 128], BF)
            make_identity(nc, ident)
            epst = singles.tile([128, 1], F32)
            nc.vector.memset(epst, 1e-5)

            # weights to SBUF
            wq_sb = pb.tile([128, NDC, DL], BF)
            wk_sb = pb.tile([128, NDC, DL], BF)
            wv_sb = pb.tile([128, NDC, DL], BF)
            wo_sb = pb.tile([128, 2, D], BF)
            bq_sb = pb.tile([128, 2], F32)
            bk_sb = pb.tile([128, 2], F32)
            nc.sync.dma_start(out=wq_sb, in_=wq)
            nc.sync.dma_start(out=wk_sb, in_=wk)
            nc.sync.dma_start(out=wv_sb, in_=wv)
            nc.sync.dma_start(out=wo_sb, in_=wo)
            nc.sync.dma_start(out=bq_sb, in_=bq)
            nc.sync.dma_start(out=bk_sb, in_=bk)

            # persistent (for phases B-D)
            qhT = pb.tile([128, 2, S], BF)      # [p, hc, s] : dh = hc*128+p
            khT = pb.tile([128, 2, S], BF)
            vh_ones = pb.tile([128, NST, HL, DK + 1], BF)  # [p, st, h, dv|1]
            xT_all = pb.tile([128, 2, S], BF)   # normalized x^T

            # ---------------- Phase A: LN + transpose ----------------
            with tc.tile_pool(name="pa", bufs=1) as pa, \
                 tc.tile_pool(name="paw", bufs=3) as paw, \
                 tc.tile_pool(name="psA", bufs=2, space="PSUM") as psA, \
                 tc.tile_pool(name="psB", bufs=3, space="PSUM") as psB:

                xnTs = {}
                for name, src in (("q", xq), ("k", xk), ("v", xv)):
                    xnT = pa.tile([128, NDC, S], BF, name=f"xnT_{name}")
                    xnTs[name] = xnT
                    for st in range(NST):
                        raw = paw.tile([128, D], BF, tag="raw")
                        nc.sync.dma_start(out=raw, in_=src[st * 128:(st + 1) * 128, :])
                        stats = paw.tile([128, 6], F32, tag="stats")
                        nc.vector.bn_stats(stats, raw)
                        mv = paw.tile([128, 2], F32, tag="mv")
                        nc.vector.bn_aggr(mv, stats)
                        std = paw.tile([128, 1], F32, tag="std")
                        nc.scalar.activation(std, mv[:, 1:2], Sqrt, bias=epst)
                        rstd = paw.tile([128, 1], F32, tag="rstd")
                        nc.vector.reciprocal(rstd, std)
                        xn = paw.tile([128, D], BF, tag="xn")
                        nc.vector.tensor_scalar(
                            out=xn, in0=raw, scalar1=mv[:, 0:1], scalar2=rstd,
                            op0=SUB, op1=MUL)
                        for dc in range(NDC):
                            ptt = psA.tile([128, 128], BF, tag="ptt")
                            nc.tensor.transpose(ptt, xn[:, dc * 128:(dc + 1) * 128], ident)
                            nc.scalar.copy(
                                out=xnT[:, dc, st * 128:(st + 1) * 128], in_=ptt)

                # ---------------- Phase B: projections ----------------
                for dst, w_sb, b_sb in ((qhT, wq_sb, bq_sb), (khT, wk_sb, bk_sb)):
                    srcT = xnTs["q"] if dst is qhT else xnTs["k"]
                    for hc in range(2):
                        for nch in range(NQC):
                            pq = psB.tile([128, QW], F32, tag="pq")
                            for dc in range(NDC):
                                nc.tensor.matmul(
                                    pq,
                                    w_sb[:, dc, hc * 128:(hc + 1) * 128],
                                    srcT[:, dc, nch * QW:(nch + 1) * QW],
                                    start=(dc == 0), stop=(dc == NDC - 1))
                            nc.vector.tensor_scalar(
                                out=dst[:, hc, nch * QW:(nch + 1) * QW], in0=pq,
                                scalar1=b_sb[:, hc:hc + 1], scalar2=None, op0=ADD)
                vnT = xnTs["v"]
                for st in range(NST):
                    pv = psB.tile([128, DL], F32, tag="pv", bufs=2)
                    for dc in range(NDC):
                        nc.tensor.matmul(
                            pv,
                            vnT[:, dc, st * 128:(st + 1) * 128],
                            wv_sb[:, dc, :],
                            start=(dc == 0), stop=(dc == NDC - 1))
                    for h in range(HL):
                        nc.scalar.copy(
                            out=vh_ones[:, st, h, 0:DK],
                            in_=pv[:, h * DK:(h + 1) * DK])
                nc.vector.memset(vh_ones[:, :, :, DK:DK + 1], 1.0)

            # ---------------- Phase C: attention ----------------
            with tc.tile_pool(name="pc", bufs=2) as pc, \
                 tc.tile_pool(name="pcs", bufs=4) as pcs, \
                 tc.tile_pool(name="psC", bufs=4, space="PSUM") as psC, \
                 tc.tile_pool(name="psX", bufs=2, space="PSUM") as psX:

                ebt_r = ebt.rearrange("(kt p) q -> p kt q", p=128)

                def emit_scores(qc, h, ebT_t, attnUT_t):
                    hc, po = h // 2, (h % 2) * 64
                    for kt in range(NKT):
                        ps_s = psC.tile([128, QW], F32, tag="s", name=f"ps_s_{qc}_{h}_{kt}")
                        nc.tensor.matmul(
                            ps_s,
                            khT[po:po + 64, hc, kt * 128:(kt + 1) * 128],
                            qhT[po:po + 64, hc, qc * QW:(qc + 1) * QW],
                            start=True, stop=True)
                        ext = pcs.tile([128, QW], BF, tag="ext", name=f"ext_{qc}_{h}_{kt}")
                        nc.scalar.activation(ext, ps_s, Exp)
                        nc.vector.tensor_tensor(
                            out=attnUT_t[:, kt, :], in0=ext, in1=ebT_t[:, kt, :], op=MUL)
                    nc.sync.dma_start(
                        out=attnu[h].rearrange("(kt p) q -> p kt q", p=128)[
                            :, :, qc * QW:(qc + 1) * QW],
                        in_=attnUT_t)

                def emit_x(qc, h, attnUT_t):
                    hc, po = h // 2, (h % 2) * 64
                    ps_x = psX.tile([65, QW], F32, tag="x", name=f"ps_x_{qc}_{h}")
                    for kt in range(NKT):
                        nc.tensor.matmul(
                            ps_x,
                            vh_ones[:, kt, h, :],
                            attnUT_t[:, kt, :],
                            start=(kt == 0), stop=(kt == NKT - 1))
                    rec = pcs.tile([1, QW], F32, tag="rec", name=f"rec_{qc}_{h}")
                    nc.vector.reciprocal(rec, ps_x[64:65, :])
                    nc.sync.dma_start(
                        out=recs[h:h + 1, qc * QW:(qc + 1) * QW], in_=rec)
                    recb = pcs.tile([64, QW], F32, tag="recb", name=f"recb_{qc}_{h}")
                    nc.sync.dma_start(
                        out=recb,
                        in_=recs[h:h + 1, qc * QW:(qc + 1) * QW].to_broadcast((64, QW)))
                    nc.vector.tensor_tensor(
                        out=xT_all[po:po + 64, hc, qc * QW:(qc + 1) * QW],
                        in0=ps_x[0:64, :], in1=recb, op=MUL)

                # 1-deep software pipeline: x-matmuls of previous (qc,h) are
                # emitted after the next scores block so PE never waits.
                prev = None
                for qc in range(NQC):
                    ebT_t = pc.tile([128, NKT, QW], BF, tag="ebT", name=f"ebT_{qc}")
                    nc.sync.dma_start(out=ebT_t, in_=ebt_r[:, :, qc * QW:(qc + 1) * QW])
                    for h in range(HL):
                        attnUT_t = pc.tile([128, NKT, QW], BF, tag="attnUT",
                                           name=f"attnUT_{qc}_{h}")
                        emit_scores(qc, h, ebT_t, attnUT_t)
                        if prev is not None:
                            emit_x(*prev)
                        prev = (qc, h, attnUT_t)
                emit_x(*prev)

            # ---------------- Phase D: output projection ----------------
            with tc.tile_pool(name="pd", bufs=3) as pd, \
                 tc.tile_pool(name="psD", bufs=4, space="PSUM") as psD:
                for mt in range(4):
                    for nch in range(NQC):
                        ps_o = psD.tile([128, QW], F32, tag="o")
                        for hc in range(2):
                            nc.tensor.matmul(
                                ps_o,
                                wo_sb[:, hc, mt * 128:(mt + 1) * 128],
                                xT_all[:, hc, nch * QW:(nch + 1) * QW],
                                start=(hc == 0), stop=(hc == 1))
                        ot = pd.tile([128, QW], F32, tag="ot")
                        nc.scalar.copy(ot, ps_o)
                        nc.sync.dma_start(
                            out=outp[mt * 128:(mt + 1) * 128, nch * QW:(nch + 1) * QW],
                            in_=ot)

    nc.compile()
    return nc


def kernel(q, k, v, mask, pos_k, ln_g, ln_b, wq, bq, wk, bk, wv, bv, wo, bo,
           layer_scale):
    global _built
    if _built is None:
        _built = _build()
    nc = _built

    f32 = np.float32
    q = np.asarray(q, f32); k = np.asarray(k, f32); v = np.asarray(v, f32)
    mask = np.asarray(mask); pos_k = np.asarray(pos_k, f32)
    ln_g = np.asarray(ln_g, f32); ln_b = np.asarray(ln_b, f32)
    wq = np.asarray(wq, f32); bq = np.asarray(bq, f32)
    wk = np.asarray(wk, f32); bk = np.asarray(bk, f32)
    wv = np.asarray(wv, f32); bv = np.asarray(bv, f32)
    wo = np.asarray(wo, f32); bo = np.asarray(bo, f32)
    layer_scale = np.asarray(layer_scale, f32)

    scale = 1.0 / np.sqrt(DK)
    # fold LN affine into the projections:  ln(x) = xc*g + b
    wq_e = (ln_g[:, None] * wq) * scale
    bq_e = (bq + ln_b @ wq) * scale
    wk_e = ln_g[:, None] * wk
    bk_e = bk + ln_b @ wk
    wv_e = ln_g[:, None] * wv
    bv_e = bv + ln_b @ wv

    # multiplicative softmax bias, transposed: [k, q]
    expb = np.exp(pos_k[:, :, 0]) * (mask != 0)
    ebt_h = np.ascontiguousarray(expb.T).astype(NPBF)

    def wlayout(w):  # [512, 256] -> [128, 4, 256]
        return np.ascontiguousarray(
            w.reshape(NDC, 128, DL).transpose(1, 0, 2)).astype(NPBF)

    in_maps = []
    for c in range(8):
        b = c // 2
        sl = slice((c % 2) * DL, (c % 2) * DL + DL)
        in_maps.append({
            "xq": q[b].astype(NPBF),
            "xk": k[b].astype(NPBF),
            "xv": v[b].astype(NPBF),
            "wq": wlayout(wq_e[:, sl]),
            "wk": wlayout(wk_e[:, sl]),
            "wv": wlayout(wv_e[:, sl]),
            "wo": np.ascontiguousarray(
                wo[sl].reshape(2, 128, D).transpose(1, 0, 2)).astype(NPBF),
            "bq": np.ascontiguousarray(bq_e[sl].reshape(2, 128).T).astype(f32),
            "bk": np.ascontiguousarray(bk_e[sl].reshape(2, 128).T).astype(f32),
            "ebt": ebt_h,
        })

    res = run_bass_kernel_spmd(nc, in_maps, list(range(8)))

    # host gather / unshard
    attn = np.empty((B, H, S, S), f32)
    out = np.empty((B, S, D), f32)
    bias_term = (bv_e @ wo + bo).astype(f32)  # rowsum(attn)=1 -> bv enters as const
    ls = layer_scale.reshape(1, D)
    for b in range(B):
        r0 = res.results[2 * b]
        r1 = res.results[2 * b + 1]
        for half, r in ((0, r0), (1, r1)):
            au = r["attnu"]            # [4, k, q] bf16, unnormalized
            rc = r["recs"]             # [4, q] f32 = 1/sums
            for hl in range(HL):
                h = half * HL + hl
                a = au[hl].astype(f32).T      # [q, k]
                a *= rc[hl][:, None]
                attn[b, h] = a
        out[b] = (r0["outp"] + r1["outp"]).T + bias_term
        out[b] *= ls
    return out, attn


# revision 6
# speedup vs baseline: 1.2080x; 1.2080x over previous
"""
MultiHeadAttention (B=4, S=2048, D=512, H=8) on 8 trn2 NeuronCores.

Sharding: core c handles batch b=c//2 and 4 heads hs=(c%2)*4..+4
(data parallel on B, tensor parallel on H).

Device kernel (per core), all matmuls bf16 with f32 PSUM accumulation:
  A) LayerNorm q,k,v (bn_stats) -> transpose to [D, S] via PE
  B) Project: qhT,khT = W.T @ xnT  -> [dh=256, S] (head-transposed),
     vh = vn @ Wv -> [S, dv] natural, with a ones column appended per head
  C) Flash-style attention in transposed score layout:
     scoresT[k,q] = khT.T@qhT (PSUM, kt-pairs) -> exp on ACT (N=1024) ->
     * expbiasT (host precomputed exp(pos_k).T * mask.T, bf16) -> attnU^T
     (bf16, stored to HBM unnormalized; host divides by sums);
     xT_plus = [vh|1].T @ attnU^T gives x^T rows + softmax sums row;
     normalize x^T columns with broadcast 1/sums.
  D) out_pT = wo_c.T @ xT_all -> [512, S] f32 partial, interleaved per
     q-chunk (host sums the two head-halves, adds biases, layer_scale).
"""

import os
import sys
from collections import deque

sys.path.insert(0, "/opt/trn_rl_repo")

import numpy as np
import ml_dtypes

import concourse.bass as bass
import concourse.bacc as bacc
import concourse.mybir as mybir
import concourse.tile as tile
from concourse.bass_utils import run_bass_kernel_spmd
from concourse.masks import make_identity

BF = mybir.dt.bfloat16
F32 = mybir.dt.float32
NPBF = ml_dtypes.bfloat16

B, S, D, H = 4, 2048, 512, 8
DK = D // H          # 64
HL = H // 2          # 4 heads per core
DL = HL * DK         # 256 local head dims
NQC = 4              # q chunks of 512
QW = S // NQC        # 512
NKT = S // 128       # 16 k tiles
NKP = NKT // 2       # 8 kt pairs
NST = S // 128       # 16 s tiles
NDC = D // 128       # 4 d chunks

_built = None


def _build():
    """Build + compile the per-core Bass program (identical on all cores)."""
    nc = bacc.Bacc("TRN2", target_bir_lowering=False, debug=False, num_devices=8)

    xq = nc.dram_tensor("xq", [S, D], BF, kind="ExternalInput").ap()
    xk = nc.dram_tensor("xk", [S, D], BF, kind="ExternalInput").ap()
    xv = nc.dram_tensor("xv", [S, D], BF, kind="ExternalInput").ap()
    wq = nc.dram_tensor("wq", [128, NDC, DL], BF, kind="ExternalInput").ap()
    wk = nc.dram_tensor("wk", [128, NDC, DL], BF, kind="ExternalInput").ap()
    wv = nc.dram_tensor("wv", [128, NDC, DL], BF, kind="ExternalInput").ap()
    wo = nc.dram_tensor("wo", [128, 2, D], BF, kind="ExternalInput").ap()
    bq = nc.dram_tensor("bq", [128, 2], F32, kind="ExternalInput").ap()
    bk = nc.dram_tensor("bk", [128, 2], F32, kind="ExternalInput").ap()
    ebt = nc.dram_tensor("ebt", [S, S], BF, kind="ExternalInput").ap()  # [k, q]

    attnu = nc.dram_tensor("attnu", [HL, S, S], BF, kind="ExternalOutput").ap()  # [h,k,q]
    sums = nc.dram_tensor("sums", [HL, S], F32, kind="ExternalOutput").ap()
    outp = nc.dram_tensor("outp", [D, S], F32, kind="ExternalOutput").ap()       # [dout,q]

    Exp = mybir.ActivationFunctionType.Exp
    Sqrt = mybir.ActivationFunctionType.Sqrt
    MUL = mybir.AluOpType.mult
    SUB = mybir.AluOpType.subtract
    ADD = mybir.AluOpType.add

    with tile.TileContext(nc) as tc:
        with tc.tile_pool(name="singles", bufs=1) as singles, \
             tc.tile_pool(name="pb", bufs=1) as pb:

            ident = singles.tile([128, 128], BF)
            make_identity(nc, ident)
            epst = singles.tile([128, 1], F32)
            nc.vector.memset(epst, 1e-5)

            # weights to SBUF
            wq_sb = pb.tile([128, NDC, DL], BF)
            wk_sb = pb.tile([128, NDC, DL], BF)
            wv_sb = pb.tile([128, NDC, DL], BF)
            wo_sb = pb.tile([128, 2, D], BF)
            bq_sb = pb.tile([128, 2], F32)
            bk_sb = pb.tile([128, 2], F32)
            nc.sync.dma_start(out=wq_sb, in_=wq)
            nc.sync.dma_start(out=wk_sb, in_=wk)
            nc.sync.dma_start(out=wv_sb, in_=wv)
            nc.sync.dma_start(out=wo_sb, in_=wo)
            nc.sync.dma_start(out=bq_sb, in_=bq)
            nc.sync.dma_start(out=bk_sb, in_=bk)

            # persistent (for phases B-D)
            qhT = pb.tile([128, 2, S], BF)      # [p, hc, s] : dh = hc*128+p
            khT = pb.tile([128, 2, S], BF)
            vh_ones = pb.tile([128, NST, HL, DK + 1], BF)  # [p, st, h, dv|1]
            xT_all = pb.tile([128, 2, S], BF)   # normalized x^T

            # ---------------- Phase A: LN + transpose ----------------
            with tc.tile_pool(name="pa", bufs=1) as pa, \
                 tc.tile_pool(name="paw", bufs=3) as paw, \
                 tc.tile_pool(name="psA", bufs=2, space="PSUM") as psA, \
                 tc.tile_pool(name="psB", bufs=2, space="PSUM") as psB:

                xnTs = {}
                for name, src in (("q", xq), ("k", xk), ("v", xv)):
                    xnT = pa.tile([128, NDC, S], BF, name=f"xnT_{name}")
                    xnTs[name] = xnT
                    for st in range(NST):
                        raw = paw.tile([128, D], BF, tag="raw")
                        nc.sync.dma_start(out=raw, in_=src[st * 128:(st + 1) * 128, :])
                        stats = paw.tile([128, 6], F32, tag="stats")
                        nc.vector.bn_stats(stats, raw)
                        mv = paw.tile([128, 2], F32, tag="mv")
                        nc.vector.bn_aggr(mv, stats)
                        std = paw.tile([128, 1], F32, tag="std")
                        nc.scalar.activation(std, mv[:, 1:2], Sqrt, bias=epst)
                        rstd = paw.tile([128, 1], F32, tag="rstd")
                        nc.vector.reciprocal(rstd, std)
                        xn = paw.tile([128, D], BF, tag="xn")
                        nc.vector.tensor_scalar(
                            out=xn, in0=raw, scalar1=mv[:, 0:1], scalar2=rstd,
                            op0=SUB, op1=MUL)
                        ptt4 = psA.tile([128, NDC, 128], BF, tag="ptt")
                        for dc in range(NDC):
                            nc.tensor.transpose(
                                ptt4[:, dc, :], xn[:, dc * 128:(dc + 1) * 128], ident)
                        nc.scalar.copy(
                            out=xnT[:, :, st * 128:(st + 1) * 128], in_=ptt4)

                # ---------------- Phase B: projections ----------------
                for dst, w_sb, b_sb, srcn in ((qhT, wq_sb, bq_sb, "q"),
                                              (khT, wk_sb, bk_sb, "k")):
                    srcT = xnTs[srcn]
                    for hc in range(2):
                        for np_ in range(2):  # nch pairs
                            pq = psB.tile([128, 2, QW], F32, tag="pq")
                            for half in range(2):
                                nch = np_ * 2 + half
                                for dc in range(NDC):
                                    nc.tensor.matmul(
                                        pq[:, half, :],
                                        w_sb[:, dc, hc * 128:(hc + 1) * 128],
                                        srcT[:, dc, nch * QW:(nch + 1) * QW],
                                        start=(dc == 0), stop=(dc == NDC - 1))
                            nc.vector.tensor_scalar(
                                out=dst[:, hc, np_ * 2 * QW:(np_ + 1) * 2 * QW],
                                in0=pq.rearrange("p a b -> p (a b)"),
                                scalar1=b_sb[:, hc:hc + 1], scalar2=None, op0=ADD)
                vnT = xnTs["v"]
                for st in range(NST):
                    pv = psB.tile([128, DL], F32, tag="pv", bufs=2)
                    for dc in range(NDC):
                        nc.tensor.matmul(
                            pv,
                            vnT[:, dc, st * 128:(st + 1) * 128],
                            wv_sb[:, dc, :],
                            start=(dc == 0), stop=(dc == NDC - 1))
                    nc.vector.tensor_copy(
                        vh_ones[:, st, :, 0:DK],
                        pv.rearrange("p (h c) -> p h c", h=HL))
                nc.vector.memset(vh_ones[:, :, :, DK:DK + 1], 1.0)

            # ------------- Phase C/D: attention + out-proj -------------
            with tc.tile_pool(name="pc", bufs=1) as pc, \
                 tc.tile_pool(name="pcs", bufs=3) as pcs, \
                 tc.tile_pool(name="pd", bufs=3) as pd, \
                 tc.tile_pool(name="psC", bufs=2, space="PSUM") as psC, \
                 tc.tile_pool(name="psX", bufs=2, space="PSUM") as psX, \
                 tc.tile_pool(name="psD", bufs=2, space="PSUM") as psD:

                ebt_r = ebt.rearrange("(kt p) q -> p kt q", p=128)

                def emit_scores(qc, h, ebT_t, attnUT_t):
                    hc, po = h // 2, (h % 2) * 64
                    for kp in range(NKP):
                        ps_s = psC.tile([128, 2, QW], F32, tag="s",
                                        name=f"ps_s_{qc}_{h}_{kp}")
                        for half in range(2):
                            kt = kp * 2 + half
                            nc.tensor.matmul(
                                ps_s[:, half, :],
                                khT[po:po + 64, hc, kt * 128:(kt + 1) * 128],
                                qhT[po:po + 64, hc, qc * QW:(qc + 1) * QW],
                                start=True, stop=True)
                        ext = pcs.tile([128, 2, QW], BF, tag="ext",
                                       name=f"ext_{qc}_{h}_{kp}")
                        nc.scalar.activation(ext, ps_s, Exp)
                        nc.vector.tensor_tensor(
                            out=attnUT_t[:, kp * 2:kp * 2 + 2, :], in0=ext,
                            in1=ebT_t[:, kp * 2:kp * 2 + 2, :], op=MUL)
                    nc.sync.dma_start(
                        out=attnu[h].rearrange("(kt p) q -> p kt q", p=128)[
                            :, :, qc * QW:(qc + 1) * QW],
                        in_=attnUT_t)

                def emit_x(qc, h, attnUT_t):
                    hc, po = h // 2, (h % 2) * 64
                    ps_x = psX.tile([65, QW], F32, tag="x", name=f"ps_x_{qc}_{h}")
                    for kt in range(NKT):
                        nc.tensor.matmul(
                            ps_x,
                            vh_ones[:, kt, h, :],
                            attnUT_t[:, kt, :],
                            start=(kt == 0), stop=(kt == NKT - 1))
                    sums_sb = pcs.tile([1, QW], F32, tag="sums", name=f"sums_{qc}_{h}")
                    nc.vector.tensor_copy(sums_sb, ps_x[64:65, :])
                    nc.sync.dma_start(
                        out=sums[h:h + 1, qc * QW:(qc + 1) * QW], in_=sums_sb)
                    sbc = pcs.tile([64, QW], F32, tag="sbc", name=f"sbc_{qc}_{h}")
                    nc.sync.dma_start(
                        out=sbc,
                        in_=sums[h:h + 1, qc * QW:(qc + 1) * QW].to_broadcast((64, QW)))
                    recb = pcs.tile([64, QW], F32, tag="recb", name=f"recb_{qc}_{h}")
                    nc.vector.reciprocal(recb, sbc)
                    nc.vector.tensor_tensor(
                        out=xT_all[po:po + 64, hc, qc * QW:(qc + 1) * QW],
                        in0=ps_x[0:64, :], in1=recb, op=MUL)

                def emit_d(qc):
                    for mt in range(4):
                        ps_o = psD.tile([128, QW], F32, tag="o",
                                        name=f"ps_o_{qc}_{mt}")
                        for hc in range(2):
                            nc.tensor.matmul(
                                ps_o,
                                wo_sb[:, hc, mt * 128:(mt + 1) * 128],
                                xT_all[:, hc, qc * QW:(qc + 1) * QW],
                                start=(hc == 0), stop=(hc == 1))
                        ot = pd.tile([128, QW], F32, tag="ot",
                                     name=f"ot_{qc}_{mt}")
                        nc.scalar.copy(ot, ps_o)
                        nc.sync.dma_start(
                            out=outp[mt * 128:(mt + 1) * 128, qc * QW:(qc + 1) * QW],
                            in_=ot)

                # software pipeline: x lags scores by 2 units; out-proj for
                # q-chunk qc emitted after x(qc+1, 0) so PE never stalls on
                # the sums round-trip.
                pend = deque()
                for qc in range(NQC):
                    ebT_t = pc.tile([128, NKT, QW], BF, tag="ebT", bufs=2,
                                    name=f"ebT_{qc}")
                    nc.sync.dma_start(out=ebT_t,
                                      in_=ebt_r[:, :, qc * QW:(qc + 1) * QW])
                    for h in range(HL):
                        attnUT_t = pc.tile([128, NKT, QW], BF, tag="attnUT", bufs=3,
                                           name=f"attnUT_{qc}_{h}")
                        emit_scores(qc, h, ebT_t, attnUT_t)
                        pend.append((qc, h, attnUT_t))
                        if len(pend) > 2:
                            u = pend.popleft()
                            emit_x(*u)
                            if u[1] == 0 and u[0] > 0:
                                emit_d(u[0] - 1)
                while pend:
                    u = pend.popleft()
                    emit_x(*u)
                    if u[1] == 0 and u[0] > 0:
                        emit_d(u[0] - 1)
                emit_d(NQC - 1)

    nc.compile()
    return nc


def kernel(q, k, v, mask, pos_k, ln_g, ln_b, wq, bq, wk, bk, wv, bv, wo, bo,
           layer_scale):
    global _built
    if _built is None:
        _built = _build()
    nc = _built

    f32 = np.float32
    q = np.asarray(q, f32); k = np.asarray(k, f32); v = np.asarray(v, f32)
    mask = np.asarray(mask); pos_k = np.asarray(pos_k, f32)
    ln_g = np.asarray(ln_g, f32); ln_b = np.asarray(ln_b, f32)
    wq = np.asarray(wq, f32); bq = np.asarray(bq, f32)
    wk = np.asarray(wk, f32); bk = np.asarray(bk, f32)
    wv = np.asarray(wv, f32); bv = np.asarray(bv, f32)
    wo = np.asarray(wo, f32); bo = np.asarray(bo, f32)
    layer_scale = np.asarray(layer_scale, f32)

    scale = 1.0 / np.sqrt(DK)
    # fold LN affine into the projections:  ln(x) = xc*g + b
    wq_e = (ln_g[:, None] * wq) * scale
    bq_e = (bq + ln_b @ wq) * scale
    wk_e = ln_g[:, None] * wk
    bk_e = bk + ln_b @ wk
    wv_e = ln_g[:, None] * wv
    bv_e = bv + ln_b @ wv

    # multiplicative softmax bias, transposed: [k, q]
    expb = np.exp(pos_k[:, :, 0]) * (mask != 0)
    ebt_h = np.ascontiguousarray(expb.T).astype(NPBF)

    def wlayout(w):  # [512, 256] -> [128, 4, 256]
        return np.ascontiguousarray(
            w.reshape(NDC, 128, DL).transpose(1, 0, 2)).astype(NPBF)

    in_maps = []
    for c in range(8):
        b = c // 2
        sl = slice((c % 2) * DL, (c % 2) * DL + DL)
        in_maps.append({
            "xq": q[b].astype(NPBF),
            "xk": k[b].astype(NPBF),
            "xv": v[b].astype(NPBF),
            "wq": wlayout(wq_e[:, sl]),
            "wk": wlayout(wk_e[:, sl]),
            "wv": wlayout(wv_e[:, sl]),
            "wo": np.ascontiguousarray(
                wo[sl].reshape(2, 128, D).transpose(1, 0, 2)).astype(NPBF),
            "bq": np.ascontiguousarray(bq_e[sl].reshape(2, 128).T).astype(f32),
            "bk": np.ascontiguousarray(bk_e[sl].reshape(2, 128).T).astype(f32),
            "ebt": ebt_h,
        })

    res = run_bass_kernel_spmd(nc, in_maps, list(range(8)))

    # host gather / unshard
    attn = np.empty((B, H, S, S), f32)
    out = np.empty((B, S, D), f32)
    bias_term = (bv_e @ wo + bo).astype(f32)  # rowsum(attn)=1 -> bv enters as const
    ls = layer_scale.reshape(1, D)
    for b in range(B):
        r0 = res.results[2 * b]
        r1 = res.results[2 * b + 1]
        for half, r in ((0, r0), (1, r1)):
            au = r["attnu"]            # [4, k, q] bf16, unnormalized
            sm = r["sums"]             # [4, q] f32
            for hl in range(HL):
                h = half * HL + hl
                a = au[hl].astype(f32).T      # [q, k]
                a /= sm[hl][:, None]
                attn[b, h] = a
        out[b] = (r0["outp"] + r1["outp"]).T + bias_term
        out[b] *= ls
    return out, attn


# revision 8
# speedup vs baseline: 1.3311x; 1.1019x over previous
"""
MultiHeadAttention (B=4, S=2048, D=512, H=8) on 8 trn2 NeuronCores.

Sharding: core c handles batch b=c//2 and 4 heads hs=(c%2)*4..+4
(data parallel on B, tensor parallel on H).

Device kernel (per core), all matmuls bf16 with f32 PSUM accumulation:
  A) LayerNorm q,k,v (bn_stats) -> transpose to [D, S] via PE
  B) Project: qhT,khT = W.T @ xnT  -> [dh=256, S] (head-transposed),
     vh = vn @ Wv -> [S, dv] natural, with a ones column appended per head
  C) Flash-style attention in transposed score layout:
     scoresT[k,q] = khT.T@qhT (PSUM, kt-pairs) -> exp on ACT (N=1024) ->
     * expbiasT (host precomputed exp(pos_k).T * mask.T, bf16) -> attnU^T
     (bf16, stored to HBM unnormalized; host divides by sums);
     xT_plus = [vh|1].T @ attnU^T gives x^T rows + softmax sums row;
     normalize x^T columns with broadcast 1/sums.
  D) out_pT = wo_c.T @ xT_all -> [512, S] f32 partial, interleaved per
     q-chunk (host sums the two head-halves, adds biases, layer_scale).
"""

import os
import sys
from collections import deque

sys.path.insert(0, "/opt/trn_rl_repo")

import numpy as np
import ml_dtypes

import concourse.bass as bass
import concourse.bacc as bacc
import concourse.mybir as mybir
import concourse.tile as tile
from concourse.bass_utils import run_bass_kernel_spmd
from concourse.masks import make_identity

BF = mybir.dt.bfloat16
F32 = mybir.dt.float32
NPBF = ml_dtypes.bfloat16

B, S, D, H = 4, 2048, 512, 8
DK = D // H          # 64
HL = H // 2          # 4 heads per core
DL = HL * DK         # 256 local head dims
NQC = 4              # q chunks of 512
QW = S // NQC        # 512
NKT = S // 128       # 16 k tiles
NKP = NKT // 2       # 8 kt pairs
NST = S // 128       # 16 s tiles
NDC = D // 128       # 4 d chunks

_built = None


def _build():
    """Build + compile the per-core Bass program (identical on all cores)."""
    nc = bacc.Bacc("TRN2", target_bir_lowering=False, debug=False, num_devices=8)

    xq = nc.dram_tensor("xq", [S, D], BF, kind="ExternalInput").ap()
    xk = nc.dram_tensor("xk", [S, D], BF, kind="ExternalInput").ap()
    xv = nc.dram_tensor("xv", [S, D], BF, kind="ExternalInput").ap()
    wq = nc.dram_tensor("wq", [128, NDC, DL], BF, kind="ExternalInput").ap()
    wk = nc.dram_tensor("wk", [128, NDC, DL], BF, kind="ExternalInput").ap()
    wv = nc.dram_tensor("wv", [128, NDC, DL], BF, kind="ExternalInput").ap()
    wo = nc.dram_tensor("wo", [128, 2, D], BF, kind="ExternalInput").ap()
    bq = nc.dram_tensor("bq", [128, 2], F32, kind="ExternalInput").ap()
    bk = nc.dram_tensor("bk", [128, 2], F32, kind="ExternalInput").ap()
    ebt = nc.dram_tensor("ebt", [S, S], BF, kind="ExternalInput").ap()  # [k, q]

    attnu = nc.dram_tensor("attnu", [HL, S, S], BF, kind="ExternalOutput").ap()  # [h,k,q]
    sums = nc.dram_tensor("sums", [HL, S], F32, kind="ExternalOutput").ap()
    outp = nc.dram_tensor("outp", [D, S], F32, kind="ExternalOutput").ap()       # [dout,q]

    Exp = mybir.ActivationFunctionType.Exp
    Rsq = mybir.ActivationFunctionType.Abs_reciprocal_sqrt
    MUL = mybir.AluOpType.mult
    SUB = mybir.AluOpType.subtract
    ADD = mybir.AluOpType.add

    with tile.TileContext(nc) as tc:
        with tc.tile_pool(name="singles", bufs=1) as singles, \
             tc.tile_pool(name="pb", bufs=1) as pb:

            ident = singles.tile([128, 128], BF)
            make_identity(nc, ident)
            epst = singles.tile([128, 1], F32)
            nc.vector.memset(epst, 1e-5)

            # weights to SBUF
            wq_sb = pb.tile([128, NDC, DL], BF)
            wk_sb = pb.tile([128, NDC, DL], BF)
            wv_sb = pb.tile([128, NDC, DL], BF)
            wo_sb = pb.tile([128, 2, D], BF)
            bq_sb = pb.tile([128, 2], F32)
            bk_sb = pb.tile([128, 2], F32)
            nc.sync.dma_start(out=wq_sb, in_=wq)
            nc.sync.dma_start(out=wk_sb, in_=wk)
            nc.sync.dma_start(out=wv_sb, in_=wv)
            nc.sync.dma_start(out=wo_sb, in_=wo)
            nc.sync.dma_start(out=bq_sb, in_=bq)
            nc.sync.dma_start(out=bk_sb, in_=bk)

            # persistent (for phases B-D)
            qhT = pb.tile([128, 2, S], BF)      # [p, hc, s] : dh = hc*128+p
            khT = pb.tile([128, 2, S], BF)
            vh_ones = pb.tile([128, NST, HL, DK + 1], BF)  # [p, st, h, dv|1]
            xT_all = pb.tile([128, 2, S], BF)   # normalized x^T

            # ---------------- Phase A: LN + transpose ----------------
            with tc.tile_pool(name="pa", bufs=1) as pa, \
                 tc.tile_pool(name="paw", bufs=3) as paw, \
                 tc.tile_pool(name="psA", bufs=2, space="PSUM") as psA, \
                 tc.tile_pool(name="psB", bufs=2, space="PSUM") as psB:

                xnTs = {}
                for name, src in (("q", xq), ("k", xk), ("v", xv)):
                    xnT = pa.tile([128, NDC, S], BF, name=f"xnT_{name}")
                    xnTs[name] = xnT
                    for st in range(NST):
                        raw = paw.tile([128, D], BF, tag="raw")
                        nc.sync.dma_start(out=raw, in_=src[st * 128:(st + 1) * 128, :])
                        stats = paw.tile([128, 6], F32, tag="stats")
                        nc.vector.bn_stats(stats, raw)
                        mv = paw.tile([128, 2], F32, tag="mv")
                        nc.vector.bn_aggr(mv, stats)
                        rstd = paw.tile([128, 1], F32, tag="rstd")
                        nc.scalar.activation(rstd, mv[:, 1:2], Rsq, bias=epst)
                        xn = paw.tile([128, D], BF, tag="xn")
                        nc.vector.tensor_scalar(
                            out=xn, in0=raw, scalar1=mv[:, 0:1], scalar2=rstd,
                            op0=SUB, op1=MUL)
                        ptt4 = psA.tile([128, NDC, 128], BF, tag="ptt")
                        for dc in range(NDC):
                            nc.tensor.transpose(
                                ptt4[:, dc, :], xn[:, dc * 128:(dc + 1) * 128], ident)
                        nc.scalar.copy(
                            out=xnT[:, :, st * 128:(st + 1) * 128], in_=ptt4)

                # ---------------- Phase B: projections ----------------
                for dst, w_sb, b_sb, srcn in ((qhT, wq_sb, bq_sb, "q"),
                                              (khT, wk_sb, bk_sb, "k")):
                    srcT = xnTs[srcn]
                    for hc in range(2):
                        for np_ in range(2):  # nch pairs
                            pq = psB.tile([128, 2, QW], F32, tag="pq")
                            for half in range(2):
                                nch = np_ * 2 + half
                                for dc in range(NDC):
                                    nc.tensor.matmul(
                                        pq[:, half, :],
                                        w_sb[:, dc, hc * 128:(hc + 1) * 128],
                                        srcT[:, dc, nch * QW:(nch + 1) * QW],
                                        start=(dc == 0), stop=(dc == NDC - 1))
                            nc.vector.tensor_scalar(
                                out=dst[:, hc, np_ * 2 * QW:(np_ + 1) * 2 * QW],
                                in0=pq.rearrange("p a b -> p (a b)"),
                                scalar1=b_sb[:, hc:hc + 1], scalar2=None, op0=ADD)
                vnT = xnTs["v"]
                for st in range(NST):
                    pv = psB.tile([128, DL], F32, tag="pv", bufs=2)
                    for dc in range(NDC):
                        nc.tensor.matmul(
                            pv,
                            vnT[:, dc, st * 128:(st + 1) * 128],
                            wv_sb[:, dc, :],
                            start=(dc == 0), stop=(dc == NDC - 1))
                    nc.vector.tensor_copy(
                        vh_ones[:, st, :, 0:DK],
                        pv.rearrange("p (h c) -> p h c", h=HL))
                nc.vector.memset(vh_ones[:, :, :, DK:DK + 1], 1.0)

            # ------------- Phase C/D: attention + out-proj -------------
            with tc.tile_pool(name="pc", bufs=1) as pc, \
                 tc.tile_pool(name="pcs", bufs=3) as pcs, \
                 tc.tile_pool(name="pd", bufs=3) as pd, \
                 tc.tile_pool(name="psC", bufs=3, space="PSUM") as psC, \
                 tc.tile_pool(name="psX", bufs=2, space="PSUM") as psX:

                ebt_r = ebt.rearrange("(kt p) q -> p kt q", p=128)

                def emit_scores(qc, h, ebT_t, attnUT_t):
                    hc, po = h // 2, (h % 2) * 64
                    for kp in range(NKP):
                        ps_s = psC.tile([128, 2, QW], F32, tag="s",
                                        name=f"ps_s_{qc}_{h}_{kp}")
                        for half in range(2):
                            kt = kp * 2 + half
                            nc.tensor.matmul(
                                ps_s[:, half, :],
                                khT[po:po + 64, hc, kt * 128:(kt + 1) * 128],
                                qhT[po:po + 64, hc, qc * QW:(qc + 1) * QW],
                                start=True, stop=True)
                        ext = pcs.tile([128, 2, QW], BF, tag="ext", bufs=4,
                                       name=f"ext_{qc}_{h}_{kp}")
                        nc.scalar.activation(ext, ps_s, Exp)
                        nc.vector.tensor_tensor(
                            out=attnUT_t[:, kp * 2:kp * 2 + 2, :], in0=ext,
                            in1=ebT_t[:, kp * 2:kp * 2 + 2, :], op=MUL)
                    nc.sync.dma_start(
                        out=attnu[h].rearrange("(kt p) q -> p kt q", p=128)[
                            :, :, qc * QW:(qc + 1) * QW],
                        in_=attnUT_t)

                def emit_x(qc, h, attnUT_t):
                    hc, po = h // 2, (h % 2) * 64
                    ps_x = psX.tile([65, QW], F32, tag="x", name=f"ps_x_{qc}_{h}")
                    for kt in range(NKT):
                        nc.tensor.matmul(
                            ps_x,
                            vh_ones[:, kt, h, :],
                            attnUT_t[:, kt, :],
                            start=(kt == 0), stop=(kt == NKT - 1))
                    sums_sb = pcs.tile([1, QW], F32, tag="sums", name=f"sums_{qc}_{h}")
                    nc.vector.tensor_copy(sums_sb, ps_x[64:65, :])
                    nc.sync.dma_start(
                        out=sums[h:h + 1, qc * QW:(qc + 1) * QW], in_=sums_sb)
                    sbc = pcs.tile([64, QW], F32, tag="sbc", name=f"sbc_{qc}_{h}")
                    nc.sync.dma_start(
                        out=sbc,
                        in_=sums[h:h + 1, qc * QW:(qc + 1) * QW].to_broadcast((64, QW)))
                    recb = pcs.tile([64, QW], F32, tag="recb", name=f"recb_{qc}_{h}")
                    nc.vector.reciprocal(recb, sbc)
                    nc.vector.tensor_tensor(
                        out=xT_all[po:po + 64, hc, qc * QW:(qc + 1) * QW],
                        in0=ps_x[0:64, :], in1=recb, op=MUL)

                def emit_d(qc):
                    for mt in range(4):
                        ps_o = psX.tile([128, QW], F32, tag="x",
                                        name=f"ps_o_{qc}_{mt}")
                        for hc in range(2):
                            nc.tensor.matmul(
                                ps_o,
                                wo_sb[:, hc, mt * 128:(mt + 1) * 128],
                                xT_all[:, hc, qc * QW:(qc + 1) * QW],
                                start=(hc == 0), stop=(hc == 1))
                        ot = pd.tile([128, QW], F32, tag="ot",
                                     name=f"ot_{qc}_{mt}")
                        nc.scalar.copy(ot, ps_o)
                        nc.sync.dma_start(
                            out=outp[mt * 128:(mt + 1) * 128, qc * QW:(qc + 1) * QW],
                            in_=ot)

                # software pipeline: x lags scores by 2 units; out-proj for
                # q-chunk qc emitted after x(qc+1, 0) so PE never stalls on
                # the sums round-trip.
                pend = deque()
                for qc in range(NQC):
                    ebT_t = pc.tile([128, NKT, QW], BF, tag="ebT", bufs=2,
                                    name=f"ebT_{qc}")
                    nc.sync.dma_start(out=ebT_t,
                                      in_=ebt_r[:, :, qc * QW:(qc + 1) * QW])
                    for h in range(HL):
                        attnUT_t = pc.tile([128, NKT, QW], BF, tag="attnUT", bufs=3,
                                           name=f"attnUT_{qc}_{h}")
                        emit_scores(qc, h, ebT_t, attnUT_t)
                        pend.append((qc, h, attnUT_t))
                        if len(pend) > 2:
                            u = pend.popleft()
                            emit_x(*u)
                            if u[1] == 0 and u[0] > 0:
                                emit_d(u[0] - 1)
                while pend:
                    u = pend.popleft()
                    emit_x(*u)
                    if u[1] == 0 and u[0] > 0:
                        emit_d(u[0] - 1)
                emit_d(NQC - 1)

    nc.compile()
    return nc


def kernel(q, k, v, mask, pos_k, ln_g, ln_b, wq, bq, wk, bk, wv, bv, wo, bo,
           layer_scale):
    global _built
    if _built is None:
        _built = _build()
    nc = _built

    f32 = np.float32
    q = np.asarray(q, f32); k = np.asarray(k, f32); v = np.asarray(v, f32)
    mask = np.asarray(mask); pos_k = np.asarray(pos_k, f32)
    ln_g = np.asarray(ln_g, f32); ln_b = np.asarray(ln_b, f32)
    wq = np.asarray(wq, f32); bq = np.asarray(bq, f32)
    wk = np.asarray(wk, f32); bk = np.asarray(bk, f32)
    wv = np.asarray(wv, f32); bv = np.asarray(bv, f32)
    wo = np.asarray(wo, f32); bo = np.asarray(bo, f32)
    layer_scale = np.asarray(layer_scale, f32)

    scale = 1.0 / np.sqrt(DK)
    # fold LN affine into the projections:  ln(x) = xc*g + b
    wq_e = (ln_g[:, None] * wq) * scale
    bq_e = (bq + ln_b @ wq) * scale
    wk_e = ln_g[:, None] * wk
    bk_e = bk + ln_b @ wk
    wv_e = ln_g[:, None] * wv
    bv_e = bv + ln_b @ wv

    # multiplicative softmax bias, transposed: [k, q]
    expb = np.exp(pos_k[:, :, 0]) * (mask != 0)
    ebt_h = np.ascontiguousarray(expb.T).astype(NPBF)

    def wlayout(w):  # [512, 256] -> [128, 4, 256]
        return np.ascontiguousarray(
            w.reshape(NDC, 128, DL).transpose(1, 0, 2)).astype(NPBF)

    in_maps = []
    for c in range(8):
        b = c // 2
        sl = slice((c % 2) * DL, (c % 2) * DL + DL)
        in_maps.append({
            "xq": q[b].astype(NPBF),
            "xk": k[b].astype(NPBF),
            "xv": v[b].astype(NPBF),
            "wq": wlayout(wq_e[:, sl]),
            "wk": wlayout(wk_e[:, sl]),
            "wv": wlayout(wv_e[:, sl]),
            "wo": np.ascontiguousarray(
                wo[sl].reshape(2, 128, D).transpose(1, 0, 2)).astype(NPBF),
            "bq": np.ascontiguousarray(bq_e[sl].reshape(2, 128).T).astype(f32),
            "bk": np.ascontiguousarray(bk_e[sl].reshape(2, 128).T).astype(f32),
            "ebt": ebt_h,
        })

    res = run_bass_kernel_spmd(nc, in_maps, list(range(8)))

    # host gather / unshard
    attn = np.empty((B, H, S, S), f32)
    out = np.empty((B, S, D), f32)
    bias_term = (bv_e @ wo + bo).astype(f32)  # rowsum(attn)=1 -> bv enters as const
    ls = layer_scale.reshape(1, D)
    for b in range(B):
        r0 = res.results[2 * b]
        r1 = res.results[2 * b + 1]
        for half, r in ((0, r0), (1, r1)):
            au = r["attnu"]            # [4, k, q] bf16, unnormalized
            sm = r["sums"]             # [4, q] f32
            for hl in range(HL):
                h = half * HL + hl
                a = au[hl].astype(f32).T      # [q, k]
                a /= sm[hl][:, None]
                attn[b, h] = a
        out[b] = (r0["outp"] + r1["outp"]).T + bias_term
        out[b] *= ls
    return out, attn
